# revision 23
# baseline (speedup 1.0000x reference)
"""CMSBlockLinear block-ELL sparse linear forward on 8 trn2 NeuronCores.

Strategy: the block-sparse weight (R=128 x K=32 active 16x16 tiles, 25%
density) is densified on the host into W^T [2048 in, 2048 out] and cast to
bf16.  The device then runs a dense matmul y^T = W^T.T @ x^T with fp32 PSUM
accumulation.  Dense-ifying costs 4x the weight FLOPs on paper, but the PE
streams N columns per matmul regardless of M, so a dense 128-wide M uses the
array 8x better than the natural M=16 sparse formulation.

Sharding (8 cores): 4-way over tokens x 2-way over output features.
Per core: x^T shard [2048, 512] bf16 (2 MB), W^T half [2048, 1024] bf16
(4 MB), out [1024, 512] bf16 (1 MB, upcast on host).

Timeline model (from ntff traces): the framework preamble blocks all queues
until ~7.2us, so the first DMA pushes can't start earlier; exec_time is
measured from ~preamble-end (~6.3us) to the end of a fixed ~7.5us teardown
semaphore sweep, so exec ~= T(last output DMA complete) + ~1.5us.  The PE
stream itself is the floor: 136 matmul-equivalents ~= 27.5us warm.

Design: x rides the Sync HWDGE ring, w the Scalar ring, chunk-0 halves
pushed first on both rings concurrently; bufs=8 pools keep the stream ~7
chunks ahead of the PE without saturating HBM (full prefetch inflates
chunk-0's completion latency).  9 N=512 warmup matmuls keep the PE busy
(HAM clock warming) from preamble-end (~7.5us) until worst-case chunk-0
data-ready (~11.1us) — any PE idle gap there risks resetting the HAM busy
window (~3us of cold stream at 1.2 GHz).  The last 3 chunks run m-major so
PSUM banks close 0.65us apart; each bank is copied (DVE/ACT alternating)
and DMA'd per-bank immediately, and the final bank is halved across both
copy engines and both DMA rings, leaving only ~0.5us copy + push + 64KB +
receipt (~2.8us) after the final matmul.
"""

import os

import numpy as np

BATCH, SEQ = 4, 512
IN_F = OUT_F = 2048
B = 16
R = 128  # output block rows
C = 128  # input block cols
KBLK = 32  # active tiles per row

TOK = BATCH * SEQ  # 2048 tokens
TOK_SHARDS = 4
OUT_SHARDS = 2
TOK_PER = TOK // TOK_SHARDS  # 512
OUT_PER = OUT_F // OUT_SHARDS  # 1024
K_CHUNKS = IN_F // 128  # 16
M_CHUNKS = OUT_PER // 128  # 8

LAST_EXEC_TIME_NS = None

_CACHE = {}


def _ensure_profile_hook():
    """Provide antenv.axon_hooks if the image lacks it, so trace=True works.

    Mirrors trn_agent_boot._ntff_profile_via_ctypes: drives NTFF capture via
    the libaxon_pjrt.so C ABI.  Also makes upload_artifacts fall back to the
    local dir when no artifact store is reachable.
    """
    import contextlib
    import ctypes
    import sys
    import types

    try:
        import antenv.axon_hooks  # noqa: F401

        return
    except ImportError:
        pass

    so_path = "/opt/axon/libaxon_pjrt.so"
    _hook = None
    if os.path.exists(so_path):
        try:
            lib = ctypes.CDLL(so_path)
            if hasattr(lib, "axon_start_nrt_profile"):
                lib.axon_start_nrt_profile.argtypes = [
                    ctypes.POINTER(ctypes.c_int64),
                    ctypes.c_size_t,
                ]
                lib.axon_start_nrt_profile.restype = ctypes.c_int64
                lib.axon_stop_nrt_profile.argtypes = [ctypes.c_char_p]
                lib.axon_stop_nrt_profile.restype = ctypes.c_int64

                @contextlib.contextmanager
                def _ntff_hook(output_dir, device_ids):
                    import jax

                    jax.devices()
                    if device_ids:
                        ids = (ctypes.c_int64 * len(device_ids))(*device_ids)
                        rc = lib.axon_start_nrt_profile(ids, len(device_ids))
                    else:
                        rc = lib.axon_start_nrt_profile(None, 0)
                    if rc != 0:
                        raise RuntimeError(f"axon_start_nrt_profile rc={rc}")
                    try:
                        yield
                    finally:
                        n = lib.axon_stop_nrt_profile(str(output_dir).encode())
                        print(f"profile: {n} file(s) -> {output_dir}", file=sys.stderr)

                _hook = _ntff_hook
        except OSError:
            pass

    mod = types.ModuleType("antenv.axon_hooks")
    mod.get_axon_ntff_profile_hook = lambda: _hook
    sys.modules["antenv.axon_hooks"] = mod

    import concourse.bass_utils as _bu

    _orig_upload = _bu.upload_artifacts

    def _safe_upload(tmpdir):
        try:
            return _orig_upload(tmpdir)
        except Exception:
            return tmpdir
    _bu.upload_artifacts = _safe_upload


M_MAJOR_K = 13  # first k-chunk of the m-major epilogue


def _build_nc():
    import concourse.mybir as mybir
    from concourse import bacc
    from concourse.tile import TileContext

    nc = bacc.Bacc("TRN2", target_bir_lowering=False)
    xT = nc.dram_tensor("xT", [IN_F, TOK_PER], mybir.dt.bfloat16, kind="ExternalInput")
    w = nc.dram_tensor("w", [IN_F, OUT_PER], mybir.dt.bfloat16, kind="ExternalInput")
    # Chunk 0 ships as fp8-e4m3 (matmuls at bf16 speed without DoubleRow):
    # half the bytes means its DMA lands ~0.5us earlier, and chunk 0 gates
    # the whole stream start.  fp8 noise on 1/16 of the contraction adds
    # ~0.9% rel err against the 2% budget.
    x8 = nc.dram_tensor("x8", [128, TOK_PER], mybir.dt.float8e4, kind="ExternalInput")
    w8 = nc.dram_tensor("w8", [128, OUT_PER], mybir.dt.float8e4, kind="ExternalInput")
    bias = nc.dram_tensor("bias", [OUT_PER], mybir.dt.float32, kind="ExternalInput")
    # y device layout: [partition, bank, token] with bank m holding output
    # features m*128+p.  1 KB contiguous per (partition, bank) DRAM run.
    y = nc.dram_tensor(
        "y", [128, M_CHUNKS * TOK_PER], mybir.dt.bfloat16, kind="ExternalOutput"
    )

    with TileContext(nc) as tc:
        with (
            tc.tile_pool(name="consts", bufs=1) as consts,
            tc.tile_pool(name="xp", bufs=8) as xp,
            tc.tile_pool(name="wp", bufs=8) as wp,
            tc.tile_pool(name="op", bufs=1) as op,
            tc.tile_pool(name="ps", bufs=1, space="PSUM") as ps,
        ):
            psums = [
                ps.tile([128, TOK_PER], mybir.dt.float32, tag=f"ps{m}", name=f"ps{m}")
                for m in range(M_CHUNKS)
            ]

            # HAM warm-up: keep the PE busy (and its activity window filling)
            # from preamble-end (~7.5us) until chunk-0 data lands (~10.5us).
            # Contents irrelevant — the first real k=0 matmul clears each
            # bank via start=True.  Sized to end right at data-ready: a gap
            # here resets the HAM busy window and costs ~3us of cold stream.
            warm = consts.tile([128, TOK_PER], mybir.dt.bfloat16)
            nc.vector.memset(warm[:, :1], 0)
            N_WARM = 8
            for i in range(N_WARM):
                nc.tensor.matmul(
                    psums[0][:],
                    warm[:, :128],
                    warm[:],
                    start=(i == 0),
                    stop=(i == N_WARM - 1),
                )

            # Per-chunk DMAs; x rides the Sync HWDGE ring, w the Scalar ring,
            # so chunk 0's first pieces push concurrently on both rings.
            # bufs=8 keeps the stream ~7 chunks ahead of the PE without
            # saturating HBM (full prefetch inflates chunk-0's completion
            # latency and starves the warmup window).
            # Chunk 0 lives in its own fp8 tiles (never recycled); chunks
            # 1-15 rotate through the bf16 pools.
            xk0 = consts.tile([128, TOK_PER], mybir.dt.float8e4, name="xk0f8")
            wk0 = consts.tile([128, OUT_PER], mybir.dt.float8e4, name="wk0f8")
            xks, wks = [xk0], [wk0]
            for k in range(1, K_CHUNKS):
                xk = xp.tile([128, TOK_PER], mybir.dt.bfloat16, name=f"xk{k}", tag="xk")
                wk = wp.tile([128, OUT_PER], mybir.dt.bfloat16, name=f"wk{k}", tag="wk")
                xks.append(xk)
                wks.append(wk)

            H = TOK_PER // 2  # 256-token half for the k=0 passes
            WH = OUT_PER // 2

            # Chunk 0 first, halves: x0 token-halves on sync, w0 out-col
            # halves on scalar.  The first real matmul needs only x0h0 +
            # w0h0 (96 KB in flight immediately after preamble).
            nc.sync.dma_start(xks[0][:, 0:H], x8[:, 0:H])
            nc.scalar.dma_start(wks[0][:, 0:WH], w8[:, 0:WH])
            nc.sync.dma_start(xks[0][:, H:TOK_PER], x8[:, H:TOK_PER])
            nc.scalar.dma_start(wks[0][:, WH:OUT_PER], w8[:, WH:OUT_PER])
            for k in range(1, K_CHUNKS):
                nc.sync.dma_start(xks[k][:], xT[k * 128 : (k + 1) * 128, :])
                nc.scalar.dma_start(wks[k][:], w[k * 128 : (k + 1) * 128, :])

            bias_sb = consts.tile([128, M_CHUNKS], mybir.dt.float32)
            nc.sync.dma_start(bias_sb[:], bias.rearrange("(m p) -> p m", p=128))

            # k=0: two half-token passes so each matmul needs only the piece
            # of chunk 0 that has already landed (w half h covers m=4h..4h+3).
            # half 0's start=True clears the bank; half 1 must not.
            for half in range(2):
                for m in range(M_CHUNKS):
                    nc.tensor.matmul(
                        psums[m][:, half * H : (half + 1) * H],
                        wks[0][:, m * 128 : (m + 1) * 128],
                        xks[0][:, half * H : (half + 1) * H],
                        start=(half == 0),
                        stop=False,
                    )

            # k=1..M_MAJOR_K-1: k-major (chunk arrives -> 8 matmuls).
            for k in range(1, M_MAJOR_K):
                for m in range(M_CHUNKS):
                    nc.tensor.matmul(
                        psums[m][:],
                        wks[k][:, m * 128 : (m + 1) * 128],
                        xks[k][:],
                        start=False,
                        stop=False,
                    )

            # Epilogue: last chunks m-major so bank m closes ~0.65us before
            # bank m+1 — copies and per-bank output DMAs overlap the stream
            # tail instead of serializing after it.
            outs = [
                op.tile([128, TOK_PER], mybir.dt.bfloat16, name=f"out{m}", tag=f"out{m}")
                for m in range(M_CHUNKS)
            ]
            for m in range(M_CHUNKS):
                for k in range(M_MAJOR_K, K_CHUNKS):
                    nc.tensor.matmul(
                        psums[m][:],
                        wks[k][:, m * 128 : (m + 1) * 128],
                        xks[k][:],
                        start=False,
                        stop=(k == K_CHUNKS - 1),
                    )
                if m < M_CHUNKS - 1:
                    # Alternate copy engines (DVE even, ACT odd).  Pushes
                    # mostly alternate rings, but bank 5's rides sync: on
                    # scalar it would sit ahead of the final bank's ACT copy
                    # in that queue and delay the critical tail.
                    if m % 2 == 0:
                        nc.vector.tensor_scalar_add(
                            outs[m][:], psums[m][:], bias_sb[:, m : m + 1]
                        )
                    else:
                        nc.scalar.activation(
                            outs[m][:],
                            psums[m][:],
                            mybir.ActivationFunctionType.Identity,
                            bias=bias_sb[:, m : m + 1],
                        )
                    eng = nc.sync if m % 2 == 0 else nc.scalar
                    eng.dma_start(y[:, m * TOK_PER : (m + 1) * TOK_PER], outs[m][:])
                else:
                    # Final bank is the whole post-stream tail: halve it
                    # across both copy engines and both DMA rings so the
                    # last copy is ~0.47us and the last transfer 64 KB.
                    nc.vector.tensor_scalar_add(
                        outs[m][:, 0:H], psums[m][:, 0:H], bias_sb[:, m : m + 1]
                    )
                    nc.scalar.activation(
                        outs[m][:, H:TOK_PER],
                        psums[m][:, H:TOK_PER],
                        mybir.ActivationFunctionType.Identity,
                        bias=bias_sb[:, m : m + 1],
                    )
                    nc.sync.dma_start(
                        y[:, m * TOK_PER : m * TOK_PER + H], outs[m][:, 0:H]
                    )
                    nc.scalar.dma_start(
                        y[:, m * TOK_PER + H : (m + 1) * TOK_PER],
                        outs[m][:, H:TOK_PER],
                    )

    nc.finalize()
    return nc


def _densify_wT(values: np.ndarray, col_indices: np.ndarray) -> np.ndarray:
    """W^T [in=2048, out=2048] with W[r*16+i, c*16+j] = values[r,k,i,j]."""
    wT = np.zeros((C, B, R, B), dtype=np.float32)  # [c, j, r, i]
    vals_t = values.transpose(0, 1, 3, 2)  # [R, K, j, i]
    r_idx = np.arange(R)
    wT[col_indices, :, r_idx[:, None], :] = vals_t
    return wT.reshape(IN_F, OUT_F)


def kernel(x, values, col_indices, bias):
    global LAST_EXEC_TIME_NS
    import ml_dtypes

    _ensure_profile_hook()
    from concourse.bass_utils import run_bass_kernel_spmd

    if "nc" not in _CACHE:
        _CACHE["nc"] = _build_nc()
    nc = _CACHE["nc"]

    bf16 = ml_dtypes.bfloat16
    f8 = ml_dtypes.float8_e4m3
    wT_f = _densify_wT(np.asarray(values), np.asarray(col_indices))
    xT_f = np.ascontiguousarray(np.asarray(x, dtype=np.float32).reshape(TOK, IN_F).T)
    wT = wT_f.astype(bf16)
    xT = xT_f.astype(bf16)
    bias_f = np.asarray(bias, dtype=np.float32)

    in_maps = []
    for core in range(8):
        t, h = divmod(core, OUT_SHARDS)
        tok_sl = slice(t * TOK_PER, (t + 1) * TOK_PER)
        out_sl = slice(h * OUT_PER, (h + 1) * OUT_PER)
        in_maps.append(
            {
                "xT": np.ascontiguousarray(xT[:, tok_sl]),
                "w": np.ascontiguousarray(wT[:, out_sl]),
                "x8": np.ascontiguousarray(xT_f[0:128, tok_sl]).astype(f8),
                "w8": np.ascontiguousarray(wT_f[0:128, out_sl]).astype(f8),
                "bias": np.ascontiguousarray(bias_f[out_sl]),
            }
        )

    res = run_bass_kernel_spmd(
        nc,
        in_maps,
        list(range(8)),
        trace=bool(os.environ.get("BASS_TRACE")),
    )
    LAST_EXEC_TIME_NS = res.exec_time_ns

    y = np.empty((TOK, OUT_F), dtype=np.float32)
    for core in range(8):
        t, h = divmod(core, OUT_SHARDS)
        # [128, 8, TOK_PER]: bank m holds features h*1024 + m*128 + p.
        y_dev = (
            res.results[core]["y"]
            .astype(np.float32)
            .reshape(128, M_CHUNKS, TOK_PER)
            .transpose(1, 0, 2)  # [m, p, t]
        )
        y_log = y_dev.reshape(OUT_PER, TOK_PER)
        y[t * TOK_PER : (t + 1) * TOK_PER, h * OUT_PER : (h + 1) * OUT_PER] = y_log.T
    return y.reshape(BATCH, SEQ, OUT_F)


# revision 26
# speedup vs baseline: 1.0418x; 1.0418x over previous
"""CMSBlockLinear block-ELL sparse linear forward on 8 trn2 NeuronCores.

Strategy: the block-sparse weight (R=128 x K=32 active 16x16 tiles, 25%
density) is densified on the host into W^T [2048 in, 2048 out] and cast to
bf16.  The device then runs a dense matmul y^T = W^T.T @ x^T with fp32 PSUM
accumulation.  Dense-ifying costs 4x the weight FLOPs on paper, but the PE
streams N columns per matmul regardless of M, so a dense 128-wide M uses the
array 8x better than the natural M=16 sparse formulation.

Sharding (8 cores): 4-way over tokens x 2-way over output features.
Per core: x^T shard [2048, 512] bf16 (2 MB), W^T half [2048, 1024] bf16
(4 MB), out [1024, 512] bf16 (1 MB, upcast on host).

Timeline model (from ntff traces): the framework preamble blocks all queues
until ~7.2us, so the first DMA pushes can't start earlier; exec_time is
measured from ~preamble-end (~6.3us) to the end of a fixed ~7.5us teardown
semaphore sweep, so exec ~= T(last output DMA complete) + ~1.5us.  The PE
stream itself is the floor: 136 matmul-equivalents ~= 27.5us warm.

Design: x rides the Sync HWDGE ring, w the Scalar ring, chunk-0 halves
pushed first on both rings concurrently; bufs=8 pools keep the stream ~7
chunks ahead of the PE without saturating HBM (full prefetch inflates
chunk-0's completion latency).  9 N=512 warmup matmuls keep the PE busy
(HAM clock warming) from preamble-end (~7.5us) until worst-case chunk-0
data-ready (~11.1us) — any PE idle gap there risks resetting the HAM busy
window (~3us of cold stream at 1.2 GHz).  The last 3 chunks run m-major so
PSUM banks close 0.65us apart; each bank is copied (DVE/ACT alternating)
and DMA'd per-bank immediately, and the final bank is halved across both
copy engines and both DMA rings, leaving only ~0.5us copy + push + 64KB +
receipt (~2.8us) after the final matmul.
"""

import os

import numpy as np

BATCH, SEQ = 4, 512
IN_F = OUT_F = 2048
B = 16
R = 128  # output block rows
C = 128  # input block cols
KBLK = 32  # active tiles per row

TOK = BATCH * SEQ  # 2048 tokens
TOK_SHARDS = 4
OUT_SHARDS = 2
TOK_PER = TOK // TOK_SHARDS  # 512
OUT_PER = OUT_F // OUT_SHARDS  # 1024
K_CHUNKS = IN_F // 128  # 16
M_CHUNKS = OUT_PER // 128  # 8

LAST_EXEC_TIME_NS = None

_CACHE = {}


def _ensure_profile_hook():
    """Provide antenv.axon_hooks if the image lacks it, so trace=True works.

    Mirrors trn_agent_boot._ntff_profile_via_ctypes: drives NTFF capture via
    the libaxon_pjrt.so C ABI.  Also makes upload_artifacts fall back to the
    local dir when no artifact store is reachable.
    """
    import contextlib
    import ctypes
    import sys
    import types

    try:
        import antenv.axon_hooks  # noqa: F401

        return
    except ImportError:
        pass

    so_path = "/opt/axon/libaxon_pjrt.so"
    _hook = None
    if os.path.exists(so_path):
        try:
            lib = ctypes.CDLL(so_path)
            if hasattr(lib, "axon_start_nrt_profile"):
                lib.axon_start_nrt_profile.argtypes = [
                    ctypes.POINTER(ctypes.c_int64),
                    ctypes.c_size_t,
                ]
                lib.axon_start_nrt_profile.restype = ctypes.c_int64
                lib.axon_stop_nrt_profile.argtypes = [ctypes.c_char_p]
                lib.axon_stop_nrt_profile.restype = ctypes.c_int64

                @contextlib.contextmanager
                def _ntff_hook(output_dir, device_ids):
                    import jax

                    jax.devices()
                    if device_ids:
                        ids = (ctypes.c_int64 * len(device_ids))(*device_ids)
                        rc = lib.axon_start_nrt_profile(ids, len(device_ids))
                    else:
                        rc = lib.axon_start_nrt_profile(None, 0)
                    if rc != 0:
                        raise RuntimeError(f"axon_start_nrt_profile rc={rc}")
                    try:
                        yield
                    finally:
                        n = lib.axon_stop_nrt_profile(str(output_dir).encode())
                        print(f"profile: {n} file(s) -> {output_dir}", file=sys.stderr)

                _hook = _ntff_hook
        except OSError:
            pass

    mod = types.ModuleType("antenv.axon_hooks")
    mod.get_axon_ntff_profile_hook = lambda: _hook
    sys.modules["antenv.axon_hooks"] = mod

    import concourse.bass_utils as _bu

    _orig_upload = _bu.upload_artifacts

    def _safe_upload(tmpdir):
        try:
            return _orig_upload(tmpdir)
        except Exception:
            return tmpdir
    _bu.upload_artifacts = _safe_upload


M_MAJOR_K = 13  # first k-chunk of the m-major epilogue


def _build_nc():
    import concourse.mybir as mybir
    from concourse import bacc
    from concourse.tile import TileContext

    nc = bacc.Bacc("TRN2", target_bir_lowering=False)
    xT = nc.dram_tensor("xT", [IN_F, TOK_PER], mybir.dt.bfloat16, kind="ExternalInput")
    w = nc.dram_tensor("w", [IN_F, OUT_PER], mybir.dt.bfloat16, kind="ExternalInput")
    bias = nc.dram_tensor("bias", [OUT_PER], mybir.dt.float32, kind="ExternalInput")
    # y device layout: [partition, bank, token] with bank m holding output
    # features m*128+p.  1 KB contiguous per (partition, bank) DRAM run.
    y = nc.dram_tensor(
        "y", [128, M_CHUNKS * TOK_PER], mybir.dt.bfloat16, kind="ExternalOutput"
    )

    with TileContext(nc) as tc:
        with (
            tc.tile_pool(name="consts", bufs=1) as consts,
            tc.tile_pool(name="xp", bufs=8) as xp,
            tc.tile_pool(name="wp", bufs=8) as wp,
            tc.tile_pool(name="op", bufs=1) as op,
            tc.tile_pool(name="ps", bufs=1, space="PSUM") as ps,
        ):
            psums = [
                ps.tile([128, TOK_PER], mybir.dt.float32, tag=f"ps{m}", name=f"ps{m}")
                for m in range(M_CHUNKS)
            ]

            # HAM warm-up: keep the PE busy (and its activity window filling)
            # from preamble-end (~7.5us) until chunk-0 data lands (~10.5us).
            # Contents irrelevant — the first real k=0 matmul clears each
            # bank via start=True.  Sized to end right at data-ready: a gap
            # here resets the HAM busy window and costs ~3us of cold stream.
            warm = consts.tile([128, TOK_PER], mybir.dt.bfloat16)
            nc.vector.memset(warm[:, :1], 0)
            N_WARM = 9
            for i in range(N_WARM):
                nc.tensor.matmul(
                    psums[0][:],
                    warm[:, :128],
                    warm[:],
                    start=(i == 0),
                    stop=(i == N_WARM - 1),
                )

            # Per-chunk DMAs; x rides the Sync HWDGE ring, w the Scalar ring,
            # so chunk 0's first pieces push concurrently on both rings.
            # bufs=8 keeps the stream ~7 chunks ahead of the PE without
            # saturating HBM (full prefetch inflates chunk-0's completion
            # latency and starves the warmup window).
            xks, wks = [], []
            for k in range(K_CHUNKS):
                xk = xp.tile([128, TOK_PER], mybir.dt.bfloat16, name=f"xk{k}", tag="xk")
                wk = wp.tile([128, OUT_PER], mybir.dt.bfloat16, name=f"wk{k}", tag="wk")
                xks.append(xk)
                wks.append(wk)

            H = TOK_PER // 2  # 256-token half for the k=0 passes
            WH = OUT_PER // 2

            # Chunk 0 first, halves: x0 token-halves on sync, w0 out-col
            # halves on scalar.  The first real matmul needs only x0h0 +
            # w0h0 (192 KB in flight immediately after preamble).
            nc.sync.dma_start(xks[0][:, 0:H], xT[0:128, 0:H])
            nc.scalar.dma_start(wks[0][:, 0:WH], w[0:128, 0:WH])
            nc.sync.dma_start(xks[0][:, H:TOK_PER], xT[0:128, H:TOK_PER])
            nc.scalar.dma_start(wks[0][:, WH:OUT_PER], w[0:128, WH:OUT_PER])
            for k in range(1, K_CHUNKS):
                nc.sync.dma_start(xks[k][:], xT[k * 128 : (k + 1) * 128, :])
                nc.scalar.dma_start(wks[k][:], w[k * 128 : (k + 1) * 128, :])

            bias_sb = consts.tile([128, M_CHUNKS], mybir.dt.float32)
            nc.sync.dma_start(bias_sb[:], bias.rearrange("(m p) -> p m", p=128))

            # k=0: two half-token passes so each matmul needs only the piece
            # of chunk 0 that has already landed (w half h covers m=4h..4h+3).
            # half 0's start=True clears the bank; half 1 must not.
            for half in range(2):
                for m in range(M_CHUNKS):
                    nc.tensor.matmul(
                        psums[m][:, half * H : (half + 1) * H],
                        wks[0][:, m * 128 : (m + 1) * 128],
                        xks[0][:, half * H : (half + 1) * H],
                        start=(half == 0),
                        stop=False,
                    )

            # k=1..M_MAJOR_K-1: k-major (chunk arrives -> 8 matmuls).
            for k in range(1, M_MAJOR_K):
                for m in range(M_CHUNKS):
                    nc.tensor.matmul(
                        psums[m][:],
                        wks[k][:, m * 128 : (m + 1) * 128],
                        xks[k][:],
                        start=False,
                        stop=False,
                    )

            # Epilogue: last chunks m-major so bank m closes ~0.65us before
            # bank m+1 — copies and per-bank output DMAs overlap the stream
            # tail instead of serializing after it.
            outs = [
                op.tile([128, TOK_PER], mybir.dt.bfloat16, name=f"out{m}", tag=f"out{m}")
                for m in range(M_CHUNKS)
            ]
            for m in range(M_CHUNKS):
                for k in range(M_MAJOR_K, K_CHUNKS):
                    nc.tensor.matmul(
                        psums[m][:],
                        wks[k][:, m * 128 : (m + 1) * 128],
                        xks[k][:],
                        start=False,
                        stop=(k == K_CHUNKS - 1),
                    )
                if m < M_CHUNKS - 1:
                    # Alternate copy engines (DVE even, ACT odd).  Pushes
                    # mostly alternate rings, but bank 5's rides sync: on
                    # scalar it would sit ahead of the final bank's ACT copy
                    # in that queue and delay the critical tail.
                    if m % 2 == 0:
                        nc.vector.tensor_scalar_add(
                            outs[m][:], psums[m][:], bias_sb[:, m : m + 1]
                        )
                    else:
                        nc.scalar.activation(
                            outs[m][:],
                            psums[m][:],
                            mybir.ActivationFunctionType.Identity,
                            bias=bias_sb[:, m : m + 1],
                        )
                    eng = nc.sync if m % 2 == 0 else nc.scalar
                    eng.dma_start(y[:, m * TOK_PER : (m + 1) * TOK_PER], outs[m][:])
                else:
                    # Final bank is the whole post-stream tail: halve it
                    # across both copy engines and both DMA rings so the
                    # last copy is ~0.47us and the last transfer 64 KB.
                    nc.vector.tensor_scalar_add(
                        outs[m][:, 0:H], psums[m][:, 0:H], bias_sb[:, m : m + 1]
                    )
                    nc.scalar.activation(
                        outs[m][:, H:TOK_PER],
                        psums[m][:, H:TOK_PER],
                        mybir.ActivationFunctionType.Identity,
                        bias=bias_sb[:, m : m + 1],
                    )
                    nc.sync.dma_start(
                        y[:, m * TOK_PER : m * TOK_PER + H], outs[m][:, 0:H]
                    )
                    nc.scalar.dma_start(
                        y[:, m * TOK_PER + H : (m + 1) * TOK_PER],
                        outs[m][:, H:TOK_PER],
                    )

    nc.finalize()
    return nc


def _densify_wT(values: np.ndarray, col_indices: np.ndarray) -> np.ndarray:
    """W^T [in=2048, out=2048] with W[r*16+i, c*16+j] = values[r,k,i,j]."""
    wT = np.zeros((C, B, R, B), dtype=np.float32)  # [c, j, r, i]
    vals_t = values.transpose(0, 1, 3, 2)  # [R, K, j, i]
    r_idx = np.arange(R)
    wT[col_indices, :, r_idx[:, None], :] = vals_t
    return wT.reshape(IN_F, OUT_F)


def kernel(x, values, col_indices, bias):
    global LAST_EXEC_TIME_NS
    import ml_dtypes

    _ensure_profile_hook()
    from concourse.bass_utils import run_bass_kernel_spmd

    if "nc" not in _CACHE:
        _CACHE["nc"] = _build_nc()
    nc = _CACHE["nc"]

    bf16 = ml_dtypes.bfloat16
    wT = _densify_wT(np.asarray(values), np.asarray(col_indices)).astype(bf16)
    xT = np.ascontiguousarray(
        np.asarray(x, dtype=np.float32).reshape(TOK, IN_F).T
    ).astype(bf16)
    bias_f = np.asarray(bias, dtype=np.float32)

    in_maps = []
    for core in range(8):
        t, h = divmod(core, OUT_SHARDS)
        in_maps.append(
            {
                "xT": np.ascontiguousarray(xT[:, t * TOK_PER : (t + 1) * TOK_PER]),
                "w": np.ascontiguousarray(wT[:, h * OUT_PER : (h + 1) * OUT_PER]),
                "bias": np.ascontiguousarray(bias_f[h * OUT_PER : (h + 1) * OUT_PER]),
            }
        )

    res = run_bass_kernel_spmd(
        nc,
        in_maps,
        list(range(8)),
        trace=bool(os.environ.get("BASS_TRACE")),
    )
    LAST_EXEC_TIME_NS = res.exec_time_ns

    y = np.empty((TOK, OUT_F), dtype=np.float32)
    for core in range(8):
        t, h = divmod(core, OUT_SHARDS)
        # [128, 8, TOK_PER]: bank m holds features h*1024 + m*128 + p.
        y_dev = (
            res.results[core]["y"]
            .astype(np.float32)
            .reshape(128, M_CHUNKS, TOK_PER)
            .transpose(1, 0, 2)  # [m, p, t]
        )
        y_log = y_dev.reshape(OUT_PER, TOK_PER)
        y[t * TOK_PER : (t + 1) * TOK_PER, h * OUT_PER : (h + 1) * OUT_PER] = y_log.T
    return y.reshape(BATCH, SEQ, OUT_F)
